# revision 6
# baseline (speedup 1.0000x reference)
"""Multi-head attention (N=4, L=2048, D=512, H=8) on 8 Trainium2 NeuronCores.

Sharding: 8 cores = 4 batches x 2 query-halves (1024 queries each). Each core
computes full K/V projections for its batch, Q projection + attention +
output projection for its query half. Output rows partition cleanly across
cores; no collectives.

Device layouts (host pre-transposes so the device does no transposes):
  xqT/xkT/xvT: [D, L*] f32     (activations transposed)
  wqT/wkT/wvT/woT: [D, D] f32  (W.T, i.e. [d_in, d_out])
  maskT: [L, LQ] f16           (attention_mask[islice,:].T)
  pad:   [L] f16               (padding_mask[n])

Pipeline per core:
  QT[d,i], KT[d,j]  (transposed) and V[j,d] (natural, fp16, with a ones
  column per head appended for the softmax denominator) via f32r matmuls.
  Per (head, j-tile): ST[j,i] = K Q^T (f32r, psum), P = exp(ST/8) (ACT,
  fp16), P *= maskC (DVE), VT[65,i] += Vaug^T P (fp16 matmul; row 64
  accumulates the softmax denominator). Then VTn = VT[0:64] / VT[64]
  (reciprocal + partition-broadcast + multiply), and out[i,:] =
  VTn^T @ WoT + bo (f32r matmuls with a rank-1 ones x bo accumulation).
"""

import numpy as np

import concourse.bass as bass
import concourse.tile as tile
from concourse import bacc, mybir
from concourse.bass_utils import run_bass_kernel_spmd

F32 = mybir.dt.float32
F32R = mybir.dt.float32r
F16 = mybir.dt.float16

N, L, D, H = 4, 2048, 512, 8
DK = D // H          # 64
NCORES = 8
LQ = L // 2          # queries per core
P = 128
DC = D // P          # 4 d-chunks
NJT = L // P         # 16 key tiles
NIT = LQ // P        # 8 query tiles per core


def r(ap):
    return ap


def build_nc():
    nc = bacc.Bacc("TRN2", target_bir_lowering=False, debug=False,
                   num_devices=NCORES)

    xqT = nc.dram_tensor("xqT", [D, LQ], F32R, kind="ExternalInput").ap()
    xkT = nc.dram_tensor("xkT", [D, L], F32R, kind="ExternalInput").ap()
    xvT = nc.dram_tensor("xvT", [D, L], F32R, kind="ExternalInput").ap()
    wqT = nc.dram_tensor("wqT", [D, D], F32R, kind="ExternalInput").ap()
    wkT = nc.dram_tensor("wkT", [D, D], F32R, kind="ExternalInput").ap()
    wvT = nc.dram_tensor("wvT", [D, D], F32R, kind="ExternalInput").ap()
    woT = nc.dram_tensor("woT", [D, D], F32R, kind="ExternalInput").ap()
    bq = nc.dram_tensor("bq", [D], F32, kind="ExternalInput").ap()
    bk = nc.dram_tensor("bk", [D], F32, kind="ExternalInput").ap()
    bv = nc.dram_tensor("bv", [D], F32R, kind="ExternalInput").ap()
    bo = nc.dram_tensor("bo", [D], F32R, kind="ExternalInput").ap()
    ones = nc.dram_tensor("ones", [P], F32R, kind="ExternalInput").ap()
    maskT = nc.dram_tensor("maskT", [L, LQ], F16, kind="ExternalInput").ap()
    pad = nc.dram_tensor("pad", [L], F32, kind="ExternalInput").ap()
    out = nc.dram_tensor("out", [LQ, D], F32, kind="ExternalOutput").ap()

    with tile.TileContext(nc) as tc, nc.allow_low_precision(
            reason="f32r outputs feed f32r matmuls; accumulation stays f32"):
        build_kernel(tc, xqT, xkT, xvT, wqT, wkT, wvT, woT,
                     bq, bk, bv, bo, ones, maskT, pad, out)
    nc.compile()
    return nc


def build_kernel(tc, xqT, xkT, xvT, wqT, wkT, wvT, woT,
                 bq, bk, bv, bo, ones, maskT, pad, out):
    nc = tc.nc
    Exp = mybir.ActivationFunctionType.Exp
    Copy = mybir.ActivationFunctionType.Copy

    with (
        tc.tile_pool(name="persist", bufs=1) as persist,
        tc.tile_pool(name="bigpersist", bufs=1) as bigpersist,
    ):
        # ---- persistent tiles --------------------------------------------
        wo_sb = persist.tile([P, DC, D], F32R, tag="wo")
        nc.sync.dma_start(out=wo_sb, in_=woT.rearrange("(c p) n -> p c n", p=P))
        bo_row = persist.tile([1, D], F32R, tag="bo")
        nc.sync.dma_start(out=bo_row, in_=bo.rearrange("(o n) -> o n", o=1))
        ones_row = persist.tile([1, P], F32R, tag="ones")
        nc.sync.dma_start(out=ones_row, in_=ones.rearrange("(o n) -> o n", o=1))

        qt_sb = bigpersist.tile([P, DC, LQ], F32R, tag="qt")
        kt_sb = bigpersist.tile([P, DC, L], F32R, tag="kt")
        # V natural [j, d], fp16, heads interleaved with a ones column after
        # each head's 64 dims: [j-tile, head, 65]
        v_sb = bigpersist.tile([P, NJT, H, DK + 1], F16, tag="v")
        nc.vector.memset(v_sb[:, :, :, DK:DK + 1], 1.0)
        # combined mask, fp16: maskC[j, jt, i] = attn_mask[i, j] * pad[j]
        maskc = bigpersist.tile([P, NJT, LQ], F16, tag="maskc")
        nc.sync.dma_start(out=maskc, in_=maskT.rearrange("(t p) i -> p t i", p=P))
        pad_sb = persist.tile([P, NJT], F32, tag="pad")
        nc.sync.dma_start(out=pad_sb, in_=pad.rearrange("(t p) -> p t", p=P))
        for jt in range(NJT):
            nc.vector.tensor_scalar_mul(
                out=maskc[:, jt, :], in0=maskc[:, jt, :],
                scalar1=pad_sb[:, jt:jt + 1])

        # ---- projections --------------------------------------------------
        with (
            tc.tile_pool(name="wproj", bufs=1) as wproj,
            tc.tile_pool(name="xstage", bufs=3) as xstage,
            tc.tile_pool(name="projps", bufs=4, space="PSUM") as projps,
        ):
            wq_sb = wproj.tile([P, DC, D], F32R, tag="wq")
            nc.sync.dma_start(out=wq_sb, in_=wqT.rearrange("(c p) n -> p c n", p=P))
            wk_sb = wproj.tile([P, DC, D], F32R, tag="wk")
            nc.sync.dma_start(out=wk_sb, in_=wkT.rearrange("(c p) n -> p c n", p=P))
            wv_sb = wproj.tile([P, DC, D], F32R, tag="wv")
            nc.sync.dma_start(out=wv_sb, in_=wvT.rearrange("(c p) n -> p c n", p=P))
            bq_col = wproj.tile([P, DC], F32, tag="bqc")
            nc.sync.dma_start(out=bq_col, in_=bq.rearrange("(c p) -> p c", p=P))
            bk_col = wproj.tile([P, DC], F32, tag="bkc")
            nc.sync.dma_start(out=bk_col, in_=bk.rearrange("(c p) -> p c", p=P))
            bv_row = wproj.tile([1, D], F32R, tag="bvr")
            nc.sync.dma_start(out=bv_row, in_=bv.rearrange("(o n) -> o n", o=1))

            # Q and K projections (transposed outputs)
            for src, w_sb, b_col, out_sb, ncols in (
                ("q", wq_sb, bq_col, qt_sb, LQ),
                ("k", wk_sb, bk_col, kt_sb, L),
            ):
                xT = xqT if src == "q" else xkT
                for jb in range(ncols // 512):
                    xt = xstage.tile([P, DC, 512], F32R, tag="xstage")
                    nc.sync.dma_start(
                        out=xt,
                        in_=xT.rearrange("(c p) m -> p c m", p=P)[:, :, jb * 512:(jb + 1) * 512])
                    for c in range(DC):
                        ps = projps.tile([P, 512], F32, tag="projps")
                        for k in range(DC):
                            nc.tensor.matmul(
                                ps, lhsT=r(w_sb[:, k, c * P:(c + 1) * P]),
                                rhs=r(xt[:, k, :]),
                                start=(k == 0), stop=(k == DC - 1))
                        nc.vector.tensor_scalar_add(
                            out=out_sb[:, c, jb * 512:(jb + 1) * 512],
                            in0=ps, scalar1=b_col[:, c:c + 1])

            # V projection (natural layout, fp16, head-interleaved)
            for jb in range(L // 512):
                xt = xstage.tile([P, DC, 512], F32R, tag="xstage")
                nc.sync.dma_start(
                    out=xt,
                    in_=xvT.rearrange("(c p) m -> p c m", p=P)[:, :, jb * 512:(jb + 1) * 512])
                for jtl in range(4):
                    jt = jb * 4 + jtl
                    ps = projps.tile([P, D], F32, tag="projpsv")
                    for k in range(DC):
                        nc.tensor.matmul(
                            ps, lhsT=r(xt[:, k, jtl * P:(jtl + 1) * P]),
                            rhs=r(wv_sb[:, k, :]),
                            start=(k == 0), stop=False)
                    nc.tensor.matmul(
                        ps, lhsT=r(ones_row), rhs=r(bv_row),
                        start=False, stop=True)
                    nc.scalar.activation(
                        out=v_sb[:, jt, :, 0:DK],
                        in_=ps.rearrange("p (h d) -> p h d", h=H), func=Copy)

        # ---- attention ----------------------------------------------------
        with (
            tc.tile_pool(name="stps", bufs=2, space="PSUM") as stps,
            tc.tile_pool(name="vtps", bufs=2, space="PSUM") as vtps,
            tc.tile_pool(name="ppool", bufs=3) as ppool,
            tc.tile_pool(name="rpool", bufs=2) as rpool,
        ):
            vtn_sb = bigpersist.tile([P, DC, LQ], F32R, tag="vtn")
            for h in range(H):
                hc, ho = h // 2, (h % 2) * DK
                vt = vtps.tile([DK + 1, LQ], F32, tag="vt")
                for jt in range(NJT):
                    st = stps.tile([P, LQ], F32, tag="st")
                    for ic in range(LQ // 512):
                        nc.tensor.matmul(
                            st[:, ic * 512:(ic + 1) * 512],
                            lhsT=r(kt_sb[ho:ho + DK, hc, jt * P:(jt + 1) * P]),
                            rhs=r(qt_sb[ho:ho + DK, hc, ic * 512:(ic + 1) * 512]),
                            start=True, stop=True)
                    p = ppool.tile([P, LQ], F16, tag="p")
                    nc.scalar.activation(out=p, in_=st, func=Exp, scale=1.0 / np.sqrt(DK))
                    nc.vector.tensor_mul(p, p, maskc[:, jt, :])
                    for ic in range(LQ // 512):
                        nc.tensor.matmul(
                            vt[:, ic * 512:(ic + 1) * 512],
                            lhsT=v_sb[:, jt, h, :],
                            rhs=p[:, ic * 512:(ic + 1) * 512],
                            start=(jt == 0), stop=(jt == NJT - 1))
                # normalize: vtn[d, i] = vt[d, i] / vt[64, i]
                rs = rpool.tile([1, LQ], F32R, tag="rs")
                nc.vector.reciprocal(out=rs, in_=vt[DK:DK + 1, :])
                # broadcast rs across 64 partitions: rank-1 matmul ones^T @ rs
                rbp = stps.tile([DK, LQ], F32, tag="st")
                for ic in range(LQ // 512):
                    nc.tensor.matmul(
                        rbp[:, ic * 512:(ic + 1) * 512],
                        lhsT=r(ones_row[:, 0:DK]),
                        rhs=r(rs[:, ic * 512:(ic + 1) * 512]),
                        start=True, stop=True)
                rb = rpool.tile([DK, LQ], F32, tag="rb")
                nc.vector.tensor_copy(out=rb, in_=rbp)
                nc.vector.tensor_mul(vtn_sb[ho:ho + DK, hc, :], vt[0:DK, :], rb)

        # ---- output projection -------------------------------------------
        with (
            tc.tile_pool(name="ops", bufs=2, space="PSUM") as ops,
            tc.tile_pool(name="obuf", bufs=3) as obuf,
        ):
            for it in range(NIT):
                po = ops.tile([P, D], F32, tag="po")
                for c in range(DC):
                    nc.tensor.matmul(
                        po, lhsT=r(vtn_sb[:, c, it * P:(it + 1) * P]),
                        rhs=r(wo_sb[:, c, :]), start=(c == 0), stop=False)
                nc.tensor.matmul(po, lhsT=r(ones_row), rhs=r(bo_row),
                                 start=False, stop=True)
                ob = obuf.tile([P, D], F32, tag="ob")
                nc.vector.tensor_copy(out=ob, in_=po)
                nc.sync.dma_start(out=out[it * P:(it + 1) * P, :], in_=ob)


_NC_CACHE = None


def _get_nc():
    global _NC_CACHE
    if _NC_CACHE is None:
        _NC_CACHE = build_nc()
    return _NC_CACHE


def make_in_maps(x_q, x_k, x_v, padding_mask, attention_mask,
                 Wq, bq, Wk, bk, Wv, bv, Wo, bo):
    f32 = np.float32
    shared = {
        "wqT": np.ascontiguousarray(np.asarray(Wq, dtype=f32).T),
        "wkT": np.ascontiguousarray(np.asarray(Wk, dtype=f32).T),
        "wvT": np.ascontiguousarray(np.asarray(Wv, dtype=f32).T),
        "woT": np.ascontiguousarray(np.asarray(Wo, dtype=f32).T),
        "bq": np.asarray(bq, dtype=f32), "bk": np.asarray(bk, dtype=f32),
        "bv": np.asarray(bv, dtype=f32), "bo": np.asarray(bo, dtype=f32),
        "ones": np.ones(P, dtype=f32),
    }
    maskT_half = [
        np.ascontiguousarray(
            np.asarray(attention_mask[half * LQ:(half + 1) * LQ, :],
                       dtype=np.float16).T)
        for half in range(2)
    ]
    xT = [np.ascontiguousarray(np.asarray(x, dtype=f32).transpose(0, 2, 1))
          for x in (x_q, x_k, x_v)]
    in_maps = []
    for core in range(NCORES):
        n, half = divmod(core, 2)
        isl = slice(half * LQ, (half + 1) * LQ)
        in_maps.append(dict(
            shared,
            xqT=np.ascontiguousarray(xT[0][n][:, isl]),
            xkT=xT[1][n],
            xvT=xT[2][n],
            maskT=maskT_half[half],
            pad=np.asarray(padding_mask[n], dtype=np.float32),
        ))
    return in_maps


def gather_out(results):
    full = np.empty((N, L, D), dtype=np.float32)
    for core in range(NCORES):
        n, half = divmod(core, 2)
        full[n, half * LQ:(half + 1) * LQ, :] = results[core]["out"]
    return full


def kernel(x_q, x_k, x_v, padding_mask, attention_mask,
           Wq, bq, Wk, bk, Wv, bv, Wo, bo):
    nc = _get_nc()
    in_maps = make_in_maps(x_q, x_k, x_v, padding_mask, attention_mask,
                           Wq, bq, Wk, bk, Wv, bv, Wo, bo)
    res = run_bass_kernel_spmd(nc, in_maps, core_ids=list(range(NCORES)))
    return gather_out(res.results)


# revision 11
# speedup vs baseline: 1.2500x; 1.2500x over previous
"""Multi-head attention (N=4, L=2048, D=512, H=8) on 8 Trainium2 NeuronCores.

Sharding: 8 cores = 4 batches x 2 query-halves (1024 queries each). Each core
computes full K/V projections for its batch, Q projection + attention +
output projection for its query half. Output rows partition cleanly across
cores; no collectives.

Device layouts (host pre-transposes so the device does no transposes):
  xqT/xkT/xvT: [D, L*] f16     (activations transposed)
  wqT/wkT/wvT/woT: [D, D] f16  (W.T, i.e. [d_in, d_out])
  maskT: [L, LQ] f16           (attention_mask[islice,:].T)
  pad:   [L] f32               (padding_mask[n])

All matmul operands are fp16 (PE streams 16-bit moving operands at full
clock; fp32 accumulate in PSUM). fp16's 10 mantissa bits match float32r's
effective precision, so this loses nothing vs the fp32r alternative while
running the PE ~2.2x faster. Q/K biases are added in f32 on the DVE.

Pipeline per core:
  QT[d,i], KT[d,j] (transposed, f16) and V[j,d] (natural, f16, ones column
  per head for the softmax denominator). Per (head, j-tile): ST[j,i] =
  K Q^T (psum f32), P = exp(ST/8) (ACT, f16), P *= maskC (DVE),
  VT[65,i] += Vaug^T P. Unnormalized VT rows and the denominator row are
  copied to SBUF per head; after all heads one batched reciprocal [8,LQ]
  computes 1/sums, which is partition-broadcast per head via a rank-1
  matmul and multiplied in (f16 2x mode). out[i,:] = VTn^T @ WoT + bo.
"""

import numpy as np

import concourse.bass as bass
import concourse.tile as tile
from concourse import bacc, mybir
from concourse.bass_utils import run_bass_kernel_spmd

F32 = mybir.dt.float32
F16 = mybir.dt.float16

N, L, D, H = 4, 2048, 512, 8
DK = D // H          # 64
NCORES = 8
LQ = L // 2          # queries per core
P = 128
DC = D // P          # 4 d-chunks
NJT = L // P         # 16 key tiles
NIT = LQ // P        # 8 query tiles per core


def build_nc():
    nc = bacc.Bacc("TRN2", target_bir_lowering=False, debug=False,
                   num_devices=NCORES)

    xqT = nc.dram_tensor("xqT", [D, LQ], F16, kind="ExternalInput").ap()
    xkT = nc.dram_tensor("xkT", [D, L], F16, kind="ExternalInput").ap()
    xvT = nc.dram_tensor("xvT", [D, L], F16, kind="ExternalInput").ap()
    wqT = nc.dram_tensor("wqT", [D, D], F16, kind="ExternalInput").ap()
    wkT = nc.dram_tensor("wkT", [D, D], F16, kind="ExternalInput").ap()
    wvT = nc.dram_tensor("wvT", [D, D], F16, kind="ExternalInput").ap()
    woT = nc.dram_tensor("woT", [D, D], F16, kind="ExternalInput").ap()
    bq = nc.dram_tensor("bq", [D], F32, kind="ExternalInput").ap()
    bk = nc.dram_tensor("bk", [D], F32, kind="ExternalInput").ap()
    bv = nc.dram_tensor("bv", [D], F16, kind="ExternalInput").ap()
    bo = nc.dram_tensor("bo", [D], F16, kind="ExternalInput").ap()
    ones = nc.dram_tensor("ones", [P], F16, kind="ExternalInput").ap()
    seld = nc.dram_tensor("seld", [H, DC * P], F16, kind="ExternalInput").ap()
    maskT = nc.dram_tensor("maskT", [L, LQ], F16, kind="ExternalInput").ap()
    pad = nc.dram_tensor("pad", [L], F32, kind="ExternalInput").ap()
    out = nc.dram_tensor("out", [LQ, D], F32, kind="ExternalOutput").ap()

    with tile.TileContext(nc) as tc, nc.allow_low_precision(
            reason="f16 matmul operands; accumulation stays f32"):
        build_kernel(tc, xqT, xkT, xvT, wqT, wkT, wvT, woT,
                     bq, bk, bv, bo, ones, seld, maskT, pad, out)
    nc.compile()
    return nc


def build_kernel(tc, xqT, xkT, xvT, wqT, wkT, wvT, woT,
                 bq, bk, bv, bo, ones, seld, maskT, pad, out):
    nc = tc.nc
    Exp = mybir.ActivationFunctionType.Exp
    Copy = mybir.ActivationFunctionType.Copy

    with (
        tc.tile_pool(name="persist", bufs=1) as persist,
        tc.tile_pool(name="bigpersist", bufs=1) as bigpersist,
    ):
        # ---- persistent tiles --------------------------------------------
        wo_sb = persist.tile([P, DC, D], F16, tag="wo")
        nc.sync.dma_start(out=wo_sb, in_=woT.rearrange("(c p) n -> p c n", p=P))
        bo_row = persist.tile([1, D], F16, tag="bo")
        nc.sync.dma_start(out=bo_row, in_=bo.rearrange("(o n) -> o n", o=1))
        ones_row = persist.tile([1, P], F16, tag="ones")
        nc.sync.dma_start(out=ones_row, in_=ones.rearrange("(o n) -> o n", o=1))

        qt_sb = bigpersist.tile([P, DC, LQ], F16, tag="qt")
        kt_sb = bigpersist.tile([P, DC, L], F16, tag="kt")
        # V natural [j, d], fp16, heads interleaved with a ones column after
        # each head's 64 dims: [j-tile, head, 65]
        v_sb = bigpersist.tile([P, NJT, H, DK + 1], F16, tag="v")
        nc.vector.memset(v_sb[:, :, :, DK:DK + 1], 1.0)
        # combined mask, fp16: maskC[j, jt, i] = attn_mask[i, j] * pad[j]
        maskc = bigpersist.tile([P, NJT, LQ], F16, tag="maskc")
        nc.sync.dma_start(out=maskc, in_=maskT.rearrange("(t p) i -> p t i", p=P))
        pad_sb = persist.tile([P, NJT], F32, tag="pad")
        nc.sync.dma_start(out=pad_sb, in_=pad.rearrange("(t p) -> p t", p=P))
        for jt in range(NJT):
            nc.vector.tensor_scalar_mul(
                out=maskc[:, jt, :], in0=maskc[:, jt, :],
                scalar1=pad_sb[:, jt:jt + 1])

        # ---- projections --------------------------------------------------
        with (
            tc.tile_pool(name="wproj", bufs=1) as wproj,
            tc.tile_pool(name="xstage", bufs=3) as xstage,
            tc.tile_pool(name="projps", bufs=4, space="PSUM") as projps,
        ):
            wq_sb = wproj.tile([P, DC, D], F16, tag="wq")
            nc.sync.dma_start(out=wq_sb, in_=wqT.rearrange("(c p) n -> p c n", p=P))
            wk_sb = wproj.tile([P, DC, D], F16, tag="wk")
            nc.sync.dma_start(out=wk_sb, in_=wkT.rearrange("(c p) n -> p c n", p=P))
            wv_sb = wproj.tile([P, DC, D], F16, tag="wv")
            nc.sync.dma_start(out=wv_sb, in_=wvT.rearrange("(c p) n -> p c n", p=P))
            bq_col = wproj.tile([P, DC], F32, tag="bqc")
            nc.sync.dma_start(out=bq_col, in_=bq.rearrange("(c p) -> p c", p=P))
            bk_col = wproj.tile([P, DC], F32, tag="bkc")
            nc.sync.dma_start(out=bk_col, in_=bk.rearrange("(c p) -> p c", p=P))
            bv_row = wproj.tile([1, D], F16, tag="bvr")
            nc.sync.dma_start(out=bv_row, in_=bv.rearrange("(o n) -> o n", o=1))

            # Q and K projections (transposed outputs)
            for src, w_sb, b_col, out_sb, ncols in (
                ("q", wq_sb, bq_col, qt_sb, LQ),
                ("k", wk_sb, bk_col, kt_sb, L),
            ):
                xT = xqT if src == "q" else xkT
                for jb in range(ncols // 512):
                    xt = xstage.tile([P, DC, 512], F16, tag="xstage")
                    nc.sync.dma_start(
                        out=xt,
                        in_=xT.rearrange("(c p) m -> p c m", p=P)[:, :, jb * 512:(jb + 1) * 512])
                    for c in range(DC):
                        ps = projps.tile([P, 512], F32, tag="projps")
                        for k in range(DC):
                            nc.tensor.matmul(
                                ps, lhsT=w_sb[:, k, c * P:(c + 1) * P],
                                rhs=xt[:, k, :],
                                start=(k == 0), stop=(k == DC - 1))
                        nc.vector.tensor_scalar_add(
                            out=out_sb[:, c, jb * 512:(jb + 1) * 512],
                            in0=ps, scalar1=b_col[:, c:c + 1])

            # V projection (natural layout, fp16, head-interleaved)
            for jb in range(L // 512):
                xt = xstage.tile([P, DC, 512], F16, tag="xstage")
                nc.sync.dma_start(
                    out=xt,
                    in_=xvT.rearrange("(c p) m -> p c m", p=P)[:, :, jb * 512:(jb + 1) * 512])
                for jtl in range(4):
                    jt = jb * 4 + jtl
                    ps = projps.tile([P, D], F32, tag="projpsv")
                    for k in range(DC):
                        nc.tensor.matmul(
                            ps, lhsT=xt[:, k, jtl * P:(jtl + 1) * P],
                            rhs=wv_sb[:, k, :],
                            start=(k == 0), stop=False)
                    nc.tensor.matmul(
                        ps, lhsT=ones_row, rhs=bv_row,
                        start=False, stop=True)
                    nc.scalar.activation(
                        out=v_sb[:, jt, :, 0:DK],
                        in_=ps.rearrange("p (h d) -> p h d", h=H), func=Copy)

        # ---- attention ----------------------------------------------------
        with (
            tc.tile_pool(name="stps", bufs=2, space="PSUM") as stps,
            tc.tile_pool(name="vtps", bufs=2, space="PSUM") as vtps,
            tc.tile_pool(name="ppool", bufs=3) as ppool,
            tc.tile_pool(name="rpool", bufs=2) as rpool,
        ):
            vtn_sb = bigpersist.tile([P, DC, LQ], F16, tag="vtn")
            sums8 = persist.tile([H, LQ], F32, tag="sums8")
            for h in range(H):
                hc, ho = h // 2, (h % 2) * DK
                vt = vtps.tile([DK + 1, LQ], F32, tag="vt")
                for jt in range(NJT):
                    st = stps.tile([P, LQ], F32, tag="st")
                    for ic in range(LQ // 512):
                        nc.tensor.matmul(
                            st[:, ic * 512:(ic + 1) * 512],
                            lhsT=kt_sb[ho:ho + DK, hc, jt * P:(jt + 1) * P],
                            rhs=qt_sb[ho:ho + DK, hc, ic * 512:(ic + 1) * 512],
                            start=True, stop=True)
                    p = ppool.tile([P, LQ], F16, tag="p")
                    nc.scalar.activation(out=p, in_=st, func=Exp,
                                         scale=1.0 / np.sqrt(DK))
                    nc.vector.tensor_mul(p, p, maskc[:, jt, :])
                    for ic in range(LQ // 512):
                        nc.tensor.matmul(
                            vt[:, ic * 512:(ic + 1) * 512],
                            lhsT=v_sb[:, jt, h, :],
                            rhs=p[:, ic * 512:(ic + 1) * 512],
                            start=(jt == 0), stop=(jt == NJT - 1))
                # stash unnormalized VT (f16) and the denominator row (f32)
                srow = rpool.tile([1, LQ], F32, tag="srow")
                nc.scalar.activation(out=srow, in_=vt[DK:DK + 1, :], func=Copy)
                nc.sync.dma_start(out=sums8[h:h + 1, :], in_=srow)
                nc.vector.tensor_copy(out=vtn_sb[ho:ho + DK, hc, :],
                                      in_=vt[0:DK, :])

            # batched normalization: one reciprocal for all heads, then per
            # d-chunk a selector matmul broadcasts 1/sum[h] to the 128
            # partitions holding heads (2c, 2c+1), followed by a f16 multiply
            sel = persist.tile([H, DC, P], F16, tag="sel")
            nc.sync.dma_start(out=sel,
                              in_=seld.rearrange("h (c p) -> h c p", p=P))
            rs8 = rpool.tile([H, LQ], F16, tag="rs8")
            nc.vector.reciprocal(out=rs8, in_=sums8)
            for c in range(DC):
                rbp = stps.tile([P, LQ], F32, tag="st")
                for ic in range(LQ // 512):
                    nc.tensor.matmul(
                        rbp[:, ic * 512:(ic + 1) * 512],
                        lhsT=sel[:, c, :],
                        rhs=rs8[:, ic * 512:(ic + 1) * 512],
                        start=True, stop=True)
                rb = rpool.tile([P, LQ], F16, tag="rb")
                nc.vector.tensor_copy(out=rb, in_=rbp)
                nc.vector.tensor_mul(vtn_sb[:, c, :], vtn_sb[:, c, :], rb)

        # ---- output projection -------------------------------------------
        with (
            tc.tile_pool(name="ops", bufs=2, space="PSUM") as ops,
            tc.tile_pool(name="obuf", bufs=3) as obuf,
        ):
            for it in range(NIT):
                po = ops.tile([P, D], F32, tag="po")
                for c in range(DC):
                    nc.tensor.matmul(
                        po, lhsT=vtn_sb[:, c, it * P:(it + 1) * P],
                        rhs=wo_sb[:, c, :], start=(c == 0), stop=False)
                nc.tensor.matmul(po, lhsT=ones_row, rhs=bo_row,
                                 start=False, stop=True)
                ob = obuf.tile([P, D], F32, tag="ob")
                nc.vector.tensor_copy(out=ob, in_=po)
                nc.sync.dma_start(out=out[it * P:(it + 1) * P, :], in_=ob)


_NC_CACHE = None


def _get_nc():
    global _NC_CACHE
    if _NC_CACHE is None:
        _NC_CACHE = build_nc()
    return _NC_CACHE


def _sel_const():
    sel = np.zeros((H, DC, P), dtype=np.float16)
    for h in range(H):
        sel[h, h // 2, (h % 2) * DK:(h % 2) * DK + DK] = 1.0
    return sel.reshape(H, DC * P)


def make_in_maps(x_q, x_k, x_v, padding_mask, attention_mask,
                 Wq, bq, Wk, bk, Wv, bv, Wo, bo):
    f16, f32 = np.float16, np.float32
    shared = {
        "wqT": np.ascontiguousarray(np.asarray(Wq, dtype=f32).T).astype(f16),
        "wkT": np.ascontiguousarray(np.asarray(Wk, dtype=f32).T).astype(f16),
        "wvT": np.ascontiguousarray(np.asarray(Wv, dtype=f32).T).astype(f16),
        "woT": np.ascontiguousarray(np.asarray(Wo, dtype=f32).T).astype(f16),
        "bq": np.asarray(bq, dtype=f32), "bk": np.asarray(bk, dtype=f32),
        "bv": np.asarray(bv, dtype=f16), "bo": np.asarray(bo, dtype=f16),
        "ones": np.ones(P, dtype=f16),
        "seld": _sel_const(),
    }
    maskT_half = [
        np.ascontiguousarray(
            np.asarray(attention_mask[half * LQ:(half + 1) * LQ, :],
                       dtype=np.float16).T)
        for half in range(2)
    ]
    xT = [np.asarray(x, dtype=f32).transpose(0, 2, 1).astype(f16)
          for x in (x_q, x_k, x_v)]
    in_maps = []
    for core in range(NCORES):
        n, half = divmod(core, 2)
        isl = slice(half * LQ, (half + 1) * LQ)
        in_maps.append(dict(
            shared,
            xqT=np.ascontiguousarray(xT[0][n][:, isl]),
            xkT=np.ascontiguousarray(xT[1][n]),
            xvT=np.ascontiguousarray(xT[2][n]),
            maskT=maskT_half[half],
            pad=np.asarray(padding_mask[n], dtype=np.float32),
        ))
    return in_maps


def gather_out(results):
    full = np.empty((N, L, D), dtype=np.float32)
    for core in range(NCORES):
        n, half = divmod(core, 2)
        full[n, half * LQ:(half + 1) * LQ, :] = results[core]["out"]
    return full


def kernel(x_q, x_k, x_v, padding_mask, attention_mask,
           Wq, bq, Wk, bk, Wv, bv, Wo, bo):
    nc = _get_nc()
    in_maps = make_in_maps(x_q, x_k, x_v, padding_mask, attention_mask,
                           Wq, bq, Wk, bk, Wv, bv, Wo, bo)
    res = run_bass_kernel_spmd(nc, in_maps, core_ids=list(range(NCORES)))
    return gather_out(res.results)
